# revision 99
# baseline (speedup 1.0000x reference)
"""Distributed Trainium2 kernel for causal GQA attention with RoPE.

Tensor-parallel over heads across 8 NeuronCores: core c owns q heads
4c..4c+3 and kv head c.  Activations are kept transposed ([dim, seq])
so every matmul contracts over the partition axis.

Schedule (single PE stream; the scalar engine's exp throughput is the
phase-2 rate limiter, so every phase-2 region pulls independent GEMM
work — later qkv chunks or o_proj blocks — through a filler queue to
keep the PE busy and the HAM clock-gate warm):

  p1(c0) p1(c1) | p2 g0,g1 (+c2 fillers) | c2 rest | p2 g2 (+c3
  fillers) | c3 rest | p2 g3 (+o_proj c0 fillers) | p3 c0..c3

Phase 1: qkvT = wqkv_c @ x.T with RoPE fused into the PSUM eviction
(vector engine); v transposed back to [seq, dim] via DMA-xbar
transposes (sync queue — bulk x loads ride the scalar HWDGE queue so
the transposes never queue behind them).

Phase 2: causal flash attention per (head, 512-row q group).  Scores
stay q-on-partitions; exp eviction on the scalar engine; row sums via
vector-engine reduces; PV accumulates O^T in PSUM with diagonal
column trimming, and the 1/rowsum normalization is applied on
eviction via a gpsimd partition-broadcast of the reciprocal row.

Phase 3: AllGather oT per 512-column group (16 collectives, each
triggered as soon as that (head, group) is evicted so the chain
overlaps phases 1-2), then out_cT = wo_c.T.T @ attn_allT accumulated
per column chunk.
"""

import math
import sys

if "/opt/trn_rl_repo" not in sys.path:
    sys.path.insert(0, "/opt/trn_rl_repo")

from contextlib import ExitStack

import numpy as np
import ml_dtypes

import concourse.bacc as bacc
import concourse.mybir as mybir
from concourse.tile import TileContext
from concourse.masks import make_identity, make_causal_mask
from concourse.bass_utils import run_bass_kernel_spmd

N_CORES = 8
H = 4096          # model dim
HD = 128          # head dim
QH = 4            # q heads per core
SCALE = 1.0 / math.sqrt(HD)
MASK_VAL = -1.0e5
DT = mybir.dt


class FillerQueue:
    """FIFO of emission thunks of guaranteed-ready PE matmuls, used to
    pad the PE stream inside scalar-bound phase-2 regions."""

    def __init__(self):
        self.items = []

    def add(self, thunk):
        self.items.append(thunk)

    def fill(self, n):
        for _ in range(min(n, len(self.items))):
            self.items.pop(0)()

    def drain(self):
        self.fill(len(self.items))


class ChainedQueue:
    """Pops from the first non-empty queue."""

    def __init__(self, *qs):
        self.qs = qs

    def fill(self, n):
        for q in self.qs:
            take = min(n, len(q.items))
            q.fill(take)
            n -= take
            if n <= 0:
                return


class SlowStart:
    """Swallows the first `skip` fill slots before passing through."""

    def __init__(self, q, skip):
        self.q = q
        self.skip = skip

    def fill(self, n):
        if self.skip > 0:
            t = min(n, self.skip)
            self.skip -= t
            n -= t
        if n > 0:
            self.q.fill(n)


def build_nc(S=2048):
    KT = H // 128           # contraction tiles for both GEMMs
    CH = 512                # phase-1/3 seq chunk
    NCH = S // CH
    QT = S // 128           # q row tiles
    GQ = 4                  # q tiles per phase-2 group
    NG = QT // GQ
    M1 = QH + 2             # phase-1 output row tiles: 4 q heads, k, v
    HKT = KT // 2           # k-tiles per x half-chunk tile

    nc = bacc.Bacc("TRN2", target_bir_lowering=False, debug=False,
                   num_devices=N_CORES)
    xT = nc.declare_dram_parameter("xT", [H, S], DT.bfloat16, isOutput=False)
    wqkvT = nc.declare_dram_parameter("wqkvT", [H, 128 * M1], DT.bfloat16,
                                      isOutput=False)
    woT = nc.declare_dram_parameter("woT", [H, 128 * QH], DT.bfloat16,
                                    isOutput=False)
    cosT = nc.declare_dram_parameter("cosT", [HD, S], DT.float32,
                                     isOutput=False)
    sinTs = nc.declare_dram_parameter("sinTs", [HD, S], DT.float32,
                                      isOutput=False)
    out = nc.declare_dram_parameter("out", [128 * QH, S], DT.float32,
                                    isOutput=True)

    with TileContext(nc) as tc, ExitStack() as ctx:
        persist = ctx.enter_context(tc.tile_pool(name="persist", bufs=1))
        cos_sb = persist.tile([HD, S], DT.float32, name="cos_sb")
        sins_sb = persist.tile([HD, S], DT.float32, name="sins_sb")
        # qk_sb[0:4] = roped qT per head, qk_sb[4] = roped kT
        qk_sb = [persist.tile([128, S], DT.bfloat16, name=f"qk{m}")
                 for m in range(QH + 1)]
        v_sb = persist.tile([128, S], DT.bfloat16, name="v_sb")
        oT_sb = [persist.tile([128, S], DT.bfloat16, name=f"oT{h}")
                 for h in range(QH)]
        # wo resident from early on so o_proj fillers can run inside the
        # last attention part
        wo_big = persist.tile([128, KT * 128 * QH], DT.bfloat16,
                              name="wo_big")
        onesb = persist.tile([1, 128], DT.bfloat16, name="onesb")
        nc.vector.memset(onesb[:], 1.0)
        onesc = persist.tile([128, 1], DT.bfloat16, name="onesc")
        nc.vector.memset(onesc[:], 1.0)
        # S^T-layout causal mask: rows = k, cols = q; fill where k > q
        maskT = persist.tile([128, 128], DT.float32, name="maskT")
        nc.gpsimd.memset(maskT[:], 0.0)
        nc.gpsimd.affine_select(
            out=maskT[:], in_=maskT[:],
            compare_op=mybir.AluOpType.is_ge, fill=MASK_VAL,
            base=0, pattern=[[1, 128]], channel_multiplier=-1)

        # collective staging (DRAM) — one gather per (head, half): the
        # chain cost is dominated by per-op latency, so fewer/bigger ops
        dpool = ctx.enter_context(tc.tile_pool(name="dramp", bufs=1,
                                               space="DRAM"))
        # one [128,1024] gather per (head, half) — the chain is latency
        # + rendezvous dominated (~15-25us per op regardless of size),
        # so 8 ops total with half 1 triggered per-head as early as
        # possible is the sweet spot
        # Every collective costs a full 8-core rendezvous (~12us) plus
        # ring bandwidth, and the ops serialize on the CC queue — so
        # merge aggressively: ONE gather for all heads' half 0, ONE for
        # heads 3/2/1's half 1 (they finish first in the reversed p2c
        # order), and two small ones for h0 (g2 early, g3 as a tiny
        # final op) so the chain's tail never gates phase 3.
        ag_inA = dpool.tile([128, QH * (S // 2)], DT.bfloat16,
                            name="ag_inA")
        ag_outA = dpool.tile([128 * N_CORES, QH * (S // 2)], DT.bfloat16,
                             name="ag_outA", addr_space="Shared")
        ag_inB = {h: dpool.tile([128, S // 2], DT.bfloat16,
                                name=f"ag_inB{h}") for h in (1, 2, 3)}
        ag_outB = {h: dpool.tile([128 * N_CORES, S // 2], DT.bfloat16,
                                 name=f"ag_outB{h}", addr_space="Shared")
                   for h in (1, 2, 3)}
        ag_in2 = {g: dpool.tile([128, CH], DT.bfloat16,
                                name=f"ag2_in0_{g}") for g in (2, 3)}
        ag_out2 = {g: dpool.tile([128 * N_CORES, CH], DT.bfloat16,
                                 name=f"ag2_out0_{g}",
                                 addr_space="Shared") for g in (2, 3)}
        B_COL = {3: 0, 2: 1, 1: 2}  # head -> column block in the B op

        # =============== phase 2 machinery ================
        def emit_stageA():
            for h in range(QH):
                nc.sync.dma_start(
                    out=ag_inA[:, h * (S // 2):(h + 1) * (S // 2)],
                    in_=oT_sb[h][:, 0:S // 2])

        def emit_collectiveA():
            # emitted OUTSIDE the phase-1 scope: its landing must not
            # join that scope's close clock (it gates the at-load SBUF
            # zones); the gpsimd queue is otherwise empty so the op
            # still fires the moment the staging DMAs land
            nc.gpsimd.collective_compute(
                "AllGather", mybir.AluOpType.bypass,
                replica_groups=[list(range(N_CORES))],
                ins=[ag_inA[:]], outs=[ag_outA[:]])

        def emit_triggerB(h):
            nc.sync.dma_start(out=ag_inB[h][:], in_=oT_sb[h][:, S // 2:S])
            nc.gpsimd.collective_compute(
                "AllGather", mybir.AluOpType.bypass,
                replica_groups=[list(range(N_CORES))],
                ins=[ag_inB[h][:]], outs=[ag_outB[h][:]])

        def emit_trigger_g0(g):
            gs = slice(g * CH, (g + 1) * CH)
            nc.sync.dma_start(out=ag_in2[g][:], in_=oT_sb[0][:, gs])
            nc.gpsimd.collective_compute(
                "AllGather", mybir.AluOpType.bypass,
                replica_groups=[list(range(N_CORES))],
                ins=[ag_in2[g][:]], outs=[ag_out2[g][:]])

        def phase2_part(p2, groups, filler=None, heads=tuple(range(QH)),
                        rate=2, post_fill=20):
            """Emit phase-2 groups for all heads (head-major).  The
            normalization/eviction of each group is deferred until after
            the NEXT group's scores+PV so the PE never head-of-line
            blocks on the racc -> rowsum -> reciprocal chain.

            NOTE: collective triggers are NOT emitted here — a gpsimd
            collective instruction completes only when the AllGather
            lands, and any instruction emitted inside this pool scope
            ends up in the scope-close vector clock that the next
            pool's released-zone reuse dep waits on.  Triggers go after
            the scope closes."""
            pending = None
            for h in heads:
                for g in groups:
                    work = emit_group(p2, h, g, filler=filler, rate=rate)
                    if pending is not None:
                        emit_norm(p2, *pending)
                    pending = (h, g, work)
            if pending is not None:
                emit_norm(p2, *pending)
            if filler is not None:
                # push a few more fillers inside the scope so the PE
                # keeps running through the scope-close drain
                filler.fill(post_fill)

        def emit_norm(p2, h, g, work):
            ot, racc = work
            # r = colsum(racc) -> 1/x -> PE outer-product broadcast ->
            # evict.  The broadcast deliberately avoids gpsimd: a
            # collective_compute blocks the gpsimd queue until the
            # AllGather lands, so anything queued behind one stalls.
            rrow_ps = p2["rrow"].tile([1, 512], DT.float32,
                                      name=f"rp{h}_{g}", tag="rrow")
            nc.tensor.matmul(rrow_ps[:], onesc[:], racc[:],
                             start=True, stop=True)
            rcp_row = p2["rsb"].tile([1, 512], DT.float32,
                                     name=f"rs{h}_{g}", tag="rsb", bufs=1)
            nc.vector.reciprocal_approx_fast(out=rcp_row[:], in_=rrow_ps[:])
            rcp_bf = p2["rsb"].tile([1, 512], DT.bfloat16,
                                    name=f"rb{h}_{g}", tag="rbf", bufs=2)
            nc.vector.tensor_copy(rcp_bf[:], rcp_row[:])
            R_ps = p2["rrow"].tile([128, 512], DT.float32,
                                   name=f"Ri{h}_{g}", tag="rrow")
            nc.tensor.matmul(R_ps[:], onesb[:], rcp_bf[:],
                             start=True, stop=True)
            # DVE reads at most one PSUM operand per instruction: bounce
            # the broadcast row block to SBUF before the normalization
            R_sb = p2["rsb"].tile([128, 512], DT.float32,
                                  name=f"Rs{h}_{g}", tag="Rsb", bufs=2)
            nc.vector.tensor_copy(R_sb[:], R_ps[:])
            nc.vector.tensor_tensor(
                out=oT_sb[h][:, 512 * g:512 * (g + 1)],
                in0=ot[:], in1=R_sb[:], op=mybir.AluOpType.mult)

        def emit_group(p2, h, g, filler=None, rate=2):
            """One (head, q-group) of causal attention.  Scores computed
            transposed (S^T: k on partitions, q on free) so the exp
            eviction lands directly in the PV operand layout — no P
            transposes anywhere.  Row sums via bf16 tile adds on the
            vector engine + one ones-matmul per group.  `filler(n)`
            emits up to n independent PE matmuls used to keep the PE
            busy while this group's exps drain."""
            jmax = GQ * (g + 1)
            qs = slice(512 * g, 512 * (g + 1))
            ot = p2["ot"].tile([128, 512], DT.float32,
                               name=f"ot{h}_{g}", tag="ot")
            racc = p2["racc"].tile([128, 512], DT.bfloat16,
                                   name=f"ra{h}_{g}", tag="racc")
            pts_list = []

            def emit_pv(j):
                d0 = j - GQ * g
                c0p = 128 * d0 if d0 > 0 else 0
                nc.tensor.matmul(ot[:, c0p:512],
                                 v_sb[:, 128 * j:128 * (j + 1)],
                                 pts_list[j][:, c0p:512],
                                 start=(j == 0), stop=(j == jmax - 1))

            pv_done = 0
            for j in range(jmax):
                d0 = j - GQ * g  # >= 0 in the diagonal region
                c0 = 128 * d0 if d0 > 0 else 0  # first valid q column
                sc = p2["sc"].tile([128, 512], DT.float32,
                                   name=f"sc{h}_{g}_{j}", tag="sc")
                nc.tensor.matmul(sc[:, c0:512],
                                 qk_sb[QH][:, 128 * j:128 * (j + 1)],
                                 qk_sb[h][:, 512 * g + c0:512 * (g + 1)],
                                 start=True, stop=True)
                if d0 >= 0:  # triangle mask on the diagonal q-subtile
                    ds = slice(128 * d0, 128 * (d0 + 1))
                    nc.vector.tensor_tensor(out=sc[:, ds], in0=sc[:, ds],
                                            in1=maskT[:],
                                            op=mybir.AluOpType.add)
                pts = p2["pts"].tile([128, 512], DT.bfloat16,
                                     name=f"pts{h}_{g}_{j}", tag="pts")
                nc.scalar.activation(pts[:, c0:512], sc[:, c0:512],
                                     mybir.ActivationFunctionType.Exp,
                                     bias=0.0, scale=SCALE)
                if j == 0:
                    nc.vector.tensor_copy(racc[:], pts[:])
                else:
                    nc.vector.tensor_tensor(out=racc[:, c0:512],
                                            in0=racc[:, c0:512],
                                            in1=pts[:, c0:512],
                                            op=mybir.AluOpType.add)
                pts_list.append(pts)
                if filler is not None:
                    filler.fill(rate)
                if j - pv_done >= 2:
                    emit_pv(pv_done)
                    pv_done += 1
            while pv_done < jmax:
                emit_pv(pv_done)
                pv_done += 1
            return ot, racc

        def open_p2(stack, name, sc_bufs=3, ot_bufs=2):
            return {
                "sc": stack.enter_context(tc.tile_pool(
                    name=f"{name}_sc", bufs=sc_bufs, space="PSUM")),
                "ot": stack.enter_context(tc.tile_pool(
                    name=f"{name}_ot", bufs=ot_bufs, space="PSUM")),
                "rrow": stack.enter_context(tc.tile_pool(
                    name=f"{name}_rr", bufs=1, space="PSUM")),
                "pts": stack.enter_context(tc.tile_pool(
                    name=f"{name}_pts", bufs=5)),
                "racc": stack.enter_context(tc.tile_pool(
                    name=f"{name}_racc", bufs=2)),
                "rsb": stack.enter_context(tc.tile_pool(
                    name=f"{name}_rsb", bufs=2)),
            }

        # ================= phase 1 ==================
        with ExitStack() as s1, nc.named_scope("phase1_qkv"):
            wq_pool = s1.enter_context(tc.tile_pool(name="wqkv", bufs=1))
            wq_big = wq_pool.tile([128, KT * 128 * M1], DT.bfloat16,
                                  name="wq_big")
            # x half-chunk tiles: [128, 16 k-tiles x 512] — 3 bufs give
            # half-chunk-deep DMA prefetch while fitting wo_big in SBUF
            xpool = s1.enter_context(tc.tile_pool(name="xpool", bufs=3))
            tmp1 = s1.enter_context(tc.tile_pool(name="tmp1", bufs=2))
            vt_pool = s1.enter_context(tc.tile_pool(name="vtp", bufs=1))
            vT_sb = vt_pool.tile([128, S], DT.bfloat16, name="vT_sb")

            def load_w(kt0, kt1, eng=None):
                (eng or nc.sync).dma_start(
                    out=wq_big[:, kt0 * 128 * M1:kt1 * 128 * M1].rearrange(
                        "p (k m) -> p k m", k=kt1 - kt0),
                    in_=wqkvT[kt0 * 128:kt1 * 128, :].rearrange(
                        "(k p) m -> p k m", p=128))

            xb_tiles = {}  # (cb, half) -> tile

            def xb_slice(cb, k):
                xb = xb_tiles[(cb, k // HKT)]
                kk = k % HKT
                return xb[:, kk * CH:(kk + 1) * CH]

            def load_x(cb, kt0, kt1, eng=None):
                """Load k-tiles [kt0, kt1) of chunk cb (within one half)."""
                half = kt0 // HKT
                assert (kt1 - 1) // HKT == half
                ns = slice(CH * cb, CH * (cb + 1))
                xb = xb_tiles[(cb, half)]
                o0 = (kt0 - half * HKT) * CH
                o1 = (kt1 - half * HKT) * CH
                (eng or nc.sync).dma_start(
                    out=xb[:, o0:o1].rearrange(
                        "p (k n) -> p k n", k=kt1 - kt0),
                    in_=xT[kt0 * 128:kt1 * 128, ns].rearrange(
                        "(k p) n -> p k n", p=128))

            def alloc_xb(cb, half):
                xb_tiles[(cb, half)] = xpool.tile(
                    [128, HKT * CH], DT.bfloat16,
                    name=f"xb_{cb}_{half}", tag="xb")

            def emit_loads0():
                # fine-grained ramp across both HWDGE queues so the
                # first matmul starts as early as possible
                alloc_xb(0, 0)
                # first (k0, m0) weight slice alone so matmul 0 starts
                # as soon as the DMA path wakes up; w/x ranges alternate
                # between the two HWDGE queues so both pipes stay busy
                # and each k-range's w and x arrive together
                nc.sync.dma_start(out=wq_big[:, 0:128],
                                  in_=wqkvT[0:128, 0:128])
                load_x(0, 0, 1, nc.scalar)
                nc.sync.dma_start(out=wq_big[:, 128:128 * M1],
                                  in_=wqkvT[0:128, 128:128 * M1])
                load_w(1, 2); load_x(0, 1, 2, nc.scalar)
                load_w(2, 4); load_x(0, 2, 4, nc.scalar)
                load_w(4, 8, nc.scalar); load_x(0, 4, 8)
                load_w(8, 12); load_x(0, 8, 12, nc.scalar)
                load_w(12, 16, nc.scalar); load_x(0, 12, 16)
                alloc_xb(0, 1)
                load_w(16, 20); load_x(0, 16, 20, nc.scalar)
                load_w(20, 24, nc.scalar); load_x(0, 20, 24)
                load_w(24, 28); load_x(0, 24, 28, nc.scalar)
                load_w(28, 32, nc.scalar); load_x(0, 28, 32)
                # cos/sin are only needed by the c0 ropes at chunk end
                nc.scalar.dma_start(out=cos_sb[:], in_=cosT[:])
                nc.sync.dma_start(out=sins_sb[:], in_=sinTs[:])

            def emit_loads_half(cb, half):
                # bulk x loads ride the scalar HWDGE queue: the sync
                # queue keeps v transposes / gather staging current.
                # NOTE: callers must place each alloc AFTER the previous
                # user of its rotating zone has emitted all its reads.
                alloc_xb(cb, half)
                for kg in range(2):
                    k0 = half * HKT + kg * (HKT // 2)
                    load_x(cb, k0, k0 + HKT // 2, nc.scalar)

            def emit_vtr(cb):
                ns = slice(CH * cb, CH * (cb + 1))
                nc.sync.dma_start_transpose(
                    out=v_sb[:, ns].rearrange("p (b c) -> p b c", c=128),
                    in_=vT_sb[:, ns])

            def emit_rope(cb, m, acc):
                ns = slice(CH * cb, CH * (cb + 1))
                if m == QH + 1:
                    nc.vector.tensor_copy(vT_sb[:, ns], acc[:])
                    emit_vtr(cb)
                    return
                # rope: out = acc*cos + swap_halves(acc)*sin_signed
                tmp = tmp1.tile([128, CH], DT.float32,
                                name=f"tmp_{cb}_{m}", tag="tmp")
                nc.vector.tensor_tensor(out=tmp[0:64, :],
                                        in0=acc[64:128, :],
                                        in1=sins_sb[0:64, ns],
                                        op=mybir.AluOpType.mult)
                nc.vector.tensor_tensor(out=tmp[64:128, :],
                                        in0=acc[0:64, :],
                                        in1=sins_sb[64:128, ns],
                                        op=mybir.AluOpType.mult)
                nc.vector.tensor_tensor(out=qk_sb[m][:, ns],
                                        in0=acc[:],
                                        in1=cos_sb[:, ns],
                                        op=mybir.AluOpType.mult)
                nc.vector.tensor_tensor(out=qk_sb[m][:, ns],
                                        in0=qk_sb[m][:, ns],
                                        in1=tmp[:],
                                        op=mybir.AluOpType.add)

            def emit_chunk(cb, acc_pool, k_outer=False):
                accs = [acc_pool.tile([128, CH], DT.float32,
                                      name=f"acc1_{cb}_{m}", tag=f"acc{m}")
                        for m in range(M1)]
                if k_outer:
                    # c0: stream k-tiles as their DMAs land
                    for k in range(KT):
                        for m in range(M1):
                            nc.tensor.matmul(accs[m][:],
                                             wq_big[:, (k * M1 + m) * 128:
                                                       (k * M1 + m + 1) * 128],
                                             xb_slice(cb, k),
                                             start=(k == 0),
                                             stop=(k == KT - 1))
                    # rope in the next chunk's consumption order so its
                    # first PSUM-bank reuse waits only one rope
                    for m in (QH + 1, QH, 0, 1, 2, 3):
                        emit_rope(cb, m, accs[m])
                else:
                    # two k-half passes: all heads consume the first x
                    # half-tile before any needs the second (which only
                    # lands partway through this chunk), then m-outer
                    # over the second half with ropes trailing one head
                    # deep.  v first (its copy + DMA transpose feed the
                    # next attention part's PV), then k, then q heads.
                    for m in (QH + 1, QH, 0, 1, 2, 3):
                        for k in range(HKT):
                            nc.tensor.matmul(accs[m][:],
                                             wq_big[:, (k * M1 + m) * 128:
                                                       (k * M1 + m + 1) * 128],
                                             xb_slice(cb, k),
                                             start=(k == 0), stop=False)
                    for m in (QH + 1, QH, 0, 1, 2, 3):
                        for k in range(HKT, KT):
                            nc.tensor.matmul(accs[m][:],
                                             wq_big[:, (k * M1 + m) * 128:
                                                       (k * M1 + m + 1) * 128],
                                             xb_slice(cb, k),
                                             start=False,
                                             stop=(k == KT - 1))
                        emit_rope(cb, m, accs[m])

            def chunk_mm(cb, m, k, acc):
                def t():
                    nc.tensor.matmul(acc[:],
                                     wq_big[:, (k * M1 + m) * 128:
                                               (k * M1 + m + 1) * 128],
                                     xb_slice(cb, k),
                                     start=(k == 0),
                                     stop=(k == KT - 1))
                    if k == KT - 1:
                        emit_rope(cb, m, acc)
                return t

            def queue_chunk(cb, fq, fill_pool, ms, k_limit=KT,
                            fq_rest=None):
                """Queue chunk cb's qkv matmuls for heads `ms` (m-serial,
                rotating fill-pool PSUM banks) into fq.  k-tiles >=
                k_limit go to fq_rest (their x half-chunk DMA is only
                safely ahead late in the consuming phase) or are emitted
                by the returned `rest()`."""
                state = {}
                for m in ms:
                    acc = fill_pool.tile([128, CH], DT.float32,
                                         name=f"fa_{cb}_{m}", tag="facc")
                    state[m] = acc
                    for k in range(min(k_limit, KT)):
                        fq.add(chunk_mm(cb, m, k, acc))
                if fq_rest is not None:
                    for m in ms:
                        for k in range(k_limit, KT):
                            fq_rest.add(chunk_mm(cb, m, k, state[m]))

                def rest():
                    fq.drain()
                    if fq_rest is not None:
                        fq_rest.drain()
                    else:
                        for m in ms:
                            for k in range(k_limit, KT):
                                chunk_mm(cb, m, k, state[m])()
                return rest

            # 2 PSUM banks rotate through all filler accumulations;
            # opened before acc1a so the first filler matmuls land in
            # fresh banks with no zone-reuse wait on chunk-1's ropes;
            # closed after the last filler chunk so phase 3 can
            # double-buffer its accumulator banks
            fill_stack = ExitStack()
            fill_pool = fill_stack.enter_context(tc.tile_pool(
                name="fillp", bufs=2, space="PSUM"))

            emit_loads0()
            emit_loads_half(1, 0)   # fresh zone 2
            with ExitStack() as sa:
                acc_pool = sa.enter_context(tc.tile_pool(
                    name="acc1a", bufs=1, space="PSUM"))
                emit_chunk(0, acc_pool, k_outer=True)
                emit_loads_half(1, 1)   # reuses (0,0)'s zone
                emit_loads_half(2, 0)   # reuses (0,1)'s zone
                emit_chunk(1, acc_pool)
                emit_loads_half(2, 1)   # reuses (1,0)'s zone
                emit_loads_half(3, 0)   # reuses (1,1)'s zone

            fq = FillerQueue()
            rest2 = queue_chunk(2, fq, fill_pool, (QH + 1, QH, 0, 1, 2, 3))
            fq.fill(10)   # cover the scope-open barrier's rope drain
            with ExitStack() as sp2, nc.named_scope("p2_a"):
                p2 = open_p2(sp2, "p2a")
                phase2_part(p2, (0, 1), filler=fq)
            emit_stageA()
            # wo loads (h-major) during the post-p2_a lull; needed by the
            # o_proj work right after the last attention part
            for h in range(QH):
                for r in range(N_CORES):
                    krow = QH * r + h
                    nc.sync.dma_start(
                        out=wo_big[:, krow * 512:(krow + 1) * 512],
                        in_=woT[krow * 128:(krow + 1) * 128, :])
            rest2()
            emit_loads_half(3, 1)   # reuses (2,0)'s zone (read by rest2)

            fq = FillerQueue()
            fq_r = FillerQueue()
            rest3 = queue_chunk(3, fq, fill_pool,
                                (QH + 1, QH, 0, 1, 2, 3), k_limit=HKT,
                                fq_rest=fq_r)
            fq.fill(10)
            with ExitStack() as sp2, nc.named_scope("p2_b"):
                p2 = open_p2(sp2, "p2b")
                phase2_part(p2, (2,), filler=ChainedQueue(fq, fq_r),
                            post_fill=32)
            rest3()
            fill_stack.close()

        at_pool = ctx.enter_context(tc.tile_pool(name="atp", bufs=8))
        osb_pool = ctx.enter_context(tc.tile_pool(name="osb", bufs=4))
        at_tiles = {}

        def emit_at_load(h, cb):
            # NOTE: a DMA emitted just before a pool-scope boundary
            # ends up in the scope-open barrier and stalls every engine
            # until it lands.  Half-1 loads ride the (idle) scalar
            # HWDGE queue: a load waiting on a late gather would
            # otherwise head-block the sync queue and starve the
            # eviction DMAs.
            at = at_pool.tile([128, N_CORES * CH], DT.bfloat16,
                              name=f"at{h}_{cb}", tag="at")
            at_tiles[(h, cb)] = at
            half, cc = cb // 2, cb % 2
            if half == 0:
                src = ag_outA[:, h * (S // 2) + cc * CH:
                              h * (S // 2) + (cc + 1) * CH]
            elif h == 0:
                src = ag_out2[cb][:]
            else:
                src = ag_outB[h][:, cc * CH:(cc + 1) * CH]
            eng = nc.sync if half == 0 else nc.scalar
            eng.dma_start(
                out=at[:].rearrange("p (r n) -> p r n", r=N_CORES),
                in_=src.rearrange("(r p) n -> p r n", p=128))

        def p3_mms(accs, h, cb, start, stop, m_list):
            at = at_tiles[(h, cb)]
            for r in range(N_CORES):
                krow = QH * r + h
                for m in m_list:
                    nc.tensor.matmul(
                        accs[m][:],
                        wo_big[:, (krow * QH + m) * 128:
                                  (krow * QH + m + 1) * 128],
                        at[:, r * CH:(r + 1) * CH],
                        start=(start and r == 0),
                        stop=(stop and r == N_CORES - 1))

        def p3_evict(accs, cb, m_list, eng=None):
            ns = slice(CH * cb, CH * (cb + 1))
            for m in m_list:
                ob = osb_pool.tile([128, 512], DT.float32,
                                   name=f"o3_{cb}_{m}", tag="o3")
                if eng is None:
                    nc.scalar.copy(ob[:], accs[m][:])
                else:
                    eng.tensor_copy(ob[:], accs[m][:])
                nc.sync.dma_start(
                    out=out[128 * m:128 * (m + 1), ns], in_=ob[:])

        # the half-0 collective (staged inside phase 1) and h0's g2
        # gather; both emitted outside the phase-1 scope so their
        # landings don't join the close clock
        emit_collectiveA()
        emit_trigger_g0(2)

        # the half-0 gathers landed during phase-2b, so chunks 0/1 of
        # the o_proj can be loaded now and used as filler inside the
        # last attention part (1 rotating PSUM bank, vector evicts);
        # the loads are gated by the phase-1 close clock (SBUF zone
        # reuse), landing a few us into the first p2c head.  Allocation
        # order h3..h0 matches the half-1 reuse order in phase 3.
        for h in (3, 2, 1, 0):
            emit_at_load(h, 0)
        for h in (3, 2, 1, 0):
            emit_at_load(h, 1)
        p3fill_stack = ExitStack()
        p3fill = p3fill_stack.enter_context(tc.tile_pool(
            name="p3fill", bufs=2, space="PSUM"))
        fq_p3 = FillerQueue()
        for cb, m in ((0, 0), (0, 1), (0, 2), (0, 3), (1, 0)):
            p3acc = p3fill.tile([128, 512], DT.float32,
                                name=f"p3f{cb}_{m}", tag="p3f")
            for h in range(QH):
                for r in range(N_CORES):
                    def t(cb=cb, m=m, h=h, r=r, acc=p3acc):
                        krow = QH * r + h
                        nc.tensor.matmul(
                            acc[:],
                            wo_big[:, (krow * QH + m) * 128:
                                      (krow * QH + m + 1) * 128],
                            at_tiles[(h, cb)][:, r * CH:(r + 1) * CH],
                            start=(h == 0 and r == 0),
                            stop=(h == QH - 1 and r == N_CORES - 1))
                        if h == QH - 1 and r == N_CORES - 1:
                            p3_evict({m: acc}, cb, (m,), eng=nc.vector)
                    fq_p3.add(t)

        # last attention part: per-head scopes so every head's half-1
        # gather triggers as early as possible (the collective chain's
        # end gates phase-3's last chunk); half the chunk-0 o_proj
        # rides along as filler so the boundaries and the exp deficit
        # stay covered without pushing the triggers later
        # heads processed 3..0 so that h3's half-1 gather (the one
        # phase-3's LAST consumers would otherwise wait ~25us/op x 3
        # extra ops for) triggers first; phase 3 walks heads 3..0 to
        # match the landing order
        # p2c heads run UNfilled: o_proj fillers would hard-couple the
        # PE stream to the half-0 collective's landing, which carries
        # 8-core rendezvous variance — a late peer then stalls all of
        # p2c.  Draining the o_proj queue after p2c instead gives the
        # collective p2c's full span as buffer.
        for i, h in enumerate((3, 2, 1, 0)):
            with ExitStack() as sp2, nc.named_scope(f"p2c{h}"):
                p2 = open_p2(sp2, f"p2c{h}", sc_bufs=3)
                phase2_part(p2, (3,), heads=(h,))
            if h == 0:
                emit_trigger_g0(3)
            else:
                emit_triggerB(h)
        fq_p3.drain()
        p3fill_stack.close()

        # ================= phase 3 ==================
        # double-buffered accumulator banks (8 PSUM banks) so chunk
        # cb+1 accumulates while chunk cb's eviction drains; evictions
        # ride the vector engine (scalar may still be draining exps)
        acc3 = ctx.enter_context(tc.tile_pool(name="acc3", bufs=2,
                                              space="PSUM"))
        with nc.named_scope("phase3_oproj"):
            # half-1 at loads issued in gather-landing order (h3 first,
            # matching the reversed p2c trigger order); the last four
            # wait on chunk-1 blocks for their SBUF zones
            for h, cb in ((3, 2), (3, 3), (2, 2), (2, 3)):
                emit_at_load(h, cb)
            # chunk 1: m0 was a filler above; finish m 1-3
            accs = {m: acc3.tile([128, 512], DT.float32,
                                 name=f"a3_1_{m}", tag=f"a3_{m}")
                    for m in (1, 2, 3)}
            for h in (3, 2, 1, 0):
                p3_mms(accs, h, 1, start=(h == 3),
                       stop=(h == 0), m_list=(1, 2, 3))
                if h == 3:
                    emit_at_load(1, 2); emit_at_load(1, 3)
                elif h == 2:
                    emit_at_load(0, 2); emit_at_load(0, 3)
            p3_evict(accs, 1, (1, 2, 3), eng=nc.vector)
            accs = [acc3.tile([128, 512], DT.float32,
                              name=f"a3_2_{m}",
                              tag=f"a3_{m}") for m in range(QH)]
            for h in (3, 2, 1, 0):
                p3_mms(accs, h, 2, start=(h == 3),
                       stop=(h == 0), m_list=range(QH))
            p3_evict(accs, 2, range(QH), eng=nc.vector)
            # last chunk m-pair-outer so the eviction DMA of the first
            # pair overlaps the second pair's accumulation
            accs = [acc3.tile([128, 512], DT.float32,
                              name=f"a3_3_{m}",
                              tag=f"a3_{m}") for m in range(QH)]
            for m_list in ((0, 1), (2, 3)):
                for h in (3, 2, 1, 0):
                    p3_mms(accs, h, 3, start=(h == 3),
                           stop=(h == 0), m_list=m_list)
                p3_evict(accs, 3, m_list, eng=nc.vector)

    nc.compile()
    return nc


def host_inputs(x, wq, wk, wv, wo, S=2048):
    """Shard + preprocess full inputs into per-core input maps."""
    bf16 = ml_dtypes.bfloat16
    xT = np.ascontiguousarray(x.reshape(S, H).T).astype(bf16)
    inv_freq = 1.0 / (500000.0 ** (np.arange(0, HD, 2, dtype=np.float32) / HD))
    t = np.arange(S, dtype=np.float32)
    emb = np.concatenate([np.outer(t, inv_freq)] * 2, axis=-1)  # [S, HD]
    cosT = np.ascontiguousarray(np.cos(emb).T).astype(np.float32)
    sinT = np.ascontiguousarray(np.sin(emb).T).astype(np.float32)
    sinTs = sinT.copy()
    sinTs[0:64] = -sinTs[0:64]  # sign-folded for the rotate_half add
    in_maps = []
    for c in range(N_CORES):
        wqkv = np.concatenate([
            wq[128 * QH * c:128 * QH * (c + 1)],
            wk[HD * c:HD * (c + 1)],
            wv[HD * c:HD * (c + 1)],
        ], axis=0)  # [768, H]
        wqkvT = np.ascontiguousarray(wqkv.T).astype(bf16)
        woT = np.ascontiguousarray(
            wo[128 * QH * c:128 * QH * (c + 1)].T).astype(bf16)
        in_maps.append({
            "xT": xT, "wqkvT": wqkvT, "woT": woT,
            "cosT": cosT, "sinTs": sinTs,
        })
    return in_maps


_NC_CACHE = {}


def _get_nc(S=2048):
    if S not in _NC_CACHE:
        _NC_CACHE[S] = build_nc(S)
    return _NC_CACHE[S]


def run(inputs, S=2048, trace=False):
    nc = _get_nc(S)
    in_maps = host_inputs(inputs["x"], inputs["wq"], inputs["wk"],
                          inputs["wv"], inputs["wo"], S=S)
    res = run_bass_kernel_spmd(nc, in_maps, list(range(N_CORES)),
                               trace=trace)
    outp = np.empty((1, S, H), dtype=np.float32)
    for c in range(N_CORES):
        outp[0, :, 128 * QH * c:128 * QH * (c + 1)] = res.results[c]["out"].T
    return outp, res


def kernel(**inputs):
    outp, _ = run(inputs, S=2048, trace=False)
    return outp
